# revision 1
# baseline (speedup 1.0000x reference)
"""Trainium2 Bass kernel for the binarized ConvNet (nn_ConvNet_81501299409071).

Data-parallel over batch: 8192 images -> 8 NeuronCores x 1024 images.

Device pipeline (feature-major: features on partitions, batch on free dim),
everything is a float32r matmul against exactly-representable +-1 Toeplitz
weight matrices; the DoReFa binarization scale E is folded into the
post-matmul activation ops (relu(acc*E + b)).

  conv1 5x5 (1->10ch):  6x2 input tiles [128=(8 rows x 16 cols), N=512],
      4 matmuls of M=120 per tile; M packed as (row-in-pair, ch, col-pair)
      so 2x2 maxpool is two full-width DVE tensor_max ops whose outputs land
      directly in conv2-ready [120=(half, ch, col), N] tiles.
  conv2 3x3 (10->20ch): per output row, 3 accumulating K=120 matmuls x 2
      output-channel halves.
  fc1 2000->50: 20 accumulating K=100 matmuls (one per conv2 relu tile).
  fc2 50->10 + log_softmax: exp/ln on ACT, partition sum / broadcast via
      tiny ones-matmuls, final subtract on DVE.
"""
import os
import numpy as np

import concourse.bass as bass
import concourse.tile as tile
from concourse import bacc, mybir
from concourse.bass_utils import run_bass_kernel_spmd

F32 = mybir.dt.float32
F32R = mybir.dt.float32r

N_CORES = 8
B_TOTAL = 8192
BC = B_TOTAL // N_CORES  # 1024 images per core
N = 512                  # batch tile (free dim / PSUM bank)
N_TILES = BC // N

LAST_EXEC_TIME_NS = None
LAST_RESULTS = None

# ---------------------------------------------------------------------------
# weight blob layout: one [128, WBLOB_COLS] f32 DRAM tensor holding every
# stationary operand and bias column. Columns assigned in this fixed order.
# ---------------------------------------------------------------------------
_off = 0
def _take(n):
    global _off
    c = _off
    _off += n
    return c

C_LHST1 = [[_take(128) for _par in range(2)] for _jp in range(2)]   # [jp][par], K=128, M=128 (pads at 60:64, 124:128)
C_LHST2 = [[[[_take(100) for _h in range(2)] for _s in range(2)]
            for _pi in range(2)] for _pyo in range(2)]  # [yo%2][pair][s][h]
C_LHSTF1 = [[_take(50) for _s in range(2)] for _yo in range(10)]    # [yo][s], K=100
C_LHSTF2 = _take(10)          # K=50
C_ONES_ROW = _take(10)        # [1,10] ones (broadcast lhsT)
C_ONES_COL = _take(1)         # [10,1] ones (partition-sum lhsT)
C_B1 = _take(1)               # [120,1]
C_B2 = [_take(1) for _s in range(2)]  # [100,1] each
C_BF1 = _take(1)              # [50,1]
C_BF2 = _take(1)              # [10,1]
WBLOB_COLS = _off


def _host_prep(inputs):
    """Binarize weights, build +-1 Toeplitz matrices + bias columns packed
    into the wblob, and the E scales."""
    w1, b1 = inputs["w1"], inputs["b1"]
    w2, b2 = inputs["w2"], inputs["b2"]
    fw1, fb1 = inputs["fw1"], inputs["fb1"]
    fw2, fb2 = inputs["fw2"], inputs["fb2"]

    scales = {
        "E1": float(np.mean(np.abs(w1))),
        "E2": float(np.mean(np.abs(w2))),
        "Ef1": float(np.mean(np.abs(fw1))),
        "Ef2": float(np.mean(np.abs(fw2))),
    }
    s1 = np.sign(w1).astype(np.float32)
    s2 = np.sign(w2).astype(np.float32)
    sf1 = np.sign(fw1).astype(np.float32)
    sf2 = np.sign(fw2).astype(np.float32)

    wb = np.zeros((128, WBLOB_COLS), np.float32)

    # conv1 Toeplitz [j2][par] (j2 = row-in-pair): M m = jp*64 + oc*6 + c
    # (jp = pooled-row pair index within the 4-row group; pads zero)
    for j2 in range(2):
        for par in range(2):
            blk = np.zeros((128, 128), np.float32)
            for jp in range(2):
                j = 2 * jp + j2
                for oc in range(10):
                    for c in range(6):
                        m = jp * 64 + oc * 6 + c
                        xo = 2 * c + par
                        for dy in range(5):
                            r = j + dy
                            for dx in range(5):
                                xi = xo + dx
                                blk[r * 16 + xi, m] = s1[oc, 0, dy, dx]
            co = C_LHST1[j2][par]
            wb[:, co:co + 128] = blk

    # conv2 Toeplitz [yo%2][pair pi][s][h]: K p = rp*64 + ci*6 + c where rp
    # is row-in-pair of pooled pair floor(yo/2)+pi; M m = oci*10 + xo
    for pyo in range(2):
        for pi in range(2):
            for s_ in range(2):
                for h in range(2):
                    blk = np.zeros((128, 100), np.float32)
                    for rp in range(2):
                        dy = 2 * pi + rp - pyo
                        if not (0 <= dy <= 2):
                            continue
                        for ci in range(10):
                            for c in range(6):
                                pp = rp * 64 + ci * 6 + c
                                xi = 6 * h + c
                                for oci in range(10):
                                    for xo in range(10):
                                        dx = xi - xo
                                        if 0 <= dx < 3:
                                            blk[pp, oci * 10 + xo] = \
                                                s2[10 * s_ + oci, ci, dy, dx]
                    co = C_LHST2[pyo][pi][s_][h]
                    wb[:, co:co + 100] = blk

    # fc1 [yo][s]: K p = oci*10+xo -> f = (10s+oci)*100 + yo*10 + xo
    for yo in range(10):
        for s in range(2):
            blk = np.zeros((100, 50), np.float32)
            for oci in range(10):
                for xo in range(10):
                    f = (10 * s + oci) * 100 + yo * 10 + xo
                    blk[oci * 10 + xo, :] = sf1[:, f]
            co = C_LHSTF1[yo][s]
            wb[0:100, co:co + 50] = blk

    wb[0:50, C_LHSTF2:C_LHSTF2 + 10] = sf2.T
    wb[0, C_ONES_ROW:C_ONES_ROW + 10] = 1.0
    wb[0:10, C_ONES_COL] = 1.0

    # bias columns, pre-divided by the accumulated binarization scales so
    # every bias+relu runs unscaled (relu(acc + b')) on any engine; the one
    # true scale Etot is applied at the logits.
    E1, E2, Ef1 = scales["E1"], scales["E2"], scales["Ef1"]
    b1v = np.zeros(128, np.float32)
    for h in range(2):
        for ci in range(10):
            b1v[h * 64 + ci * 6:h * 64 + ci * 6 + 6] = b1[ci] / E1
    wb[:, C_B1] = b1v
    for s in range(2):
        b2v = np.repeat(b2[10 * s:10 * s + 10], 10).astype(np.float32)
        wb[0:100, C_B2[s]] = b2v / (E1 * E2)
    wb[0:50, C_BF1] = fb1 / (E1 * E2 * Ef1)
    wb[0:10, C_BF2] = fb2
    return wb, scales


# tuning knobs (engine splits / pool sizing), overridable for sweeps
CFG = {
    "rowmax_pool_mod": 3,   # rowmax i -> gpsimd when i % mod < thr
    "rowmax_pool_thr": 2,
    "cme_dve_mod": 6,       # u-extract i -> DVE when i % mod == mod-1, else ACT
    "a2_dve_mod": 4,        # a2 relu -> DVE when (2*yo+s) % mod == mod-1
    "p1_bufs": 2,
    "rhs_bufs": 8,
}


def build_program(scales, n_tiles=N_TILES, bc=BC, cfg=None, repeat=1):
    """Build the single-core SPMD bass program."""
    cfg = {**CFG, **(cfg or {})}
    Etot = scales["E1"] * scales["E2"] * scales["Ef1"] * scales["Ef2"]
    Relu = mybir.ActivationFunctionType.Relu
    Ident = mybir.ActivationFunctionType.Identity
    Exp = mybir.ActivationFunctionType.Exp
    Ln = mybir.ActivationFunctionType.Ln
    Add = mybir.AluOpType.add
    Max = mybir.AluOpType.max

    nc = bacc.Bacc("TRN2", target_bir_lowering=False, debug=False)
    # declared float32r so fp32r matmuls may consume them directly (walrus
    # requires fp32r operands to come from fp32r-emitting producers); host
    # supplies plain fp32 bits
    xT = nc.dram_tensor("xT", [28, 28, bc], F32R, kind="ExternalInput").ap()
    wblob = nc.dram_tensor("wblob", [128, WBLOB_COLS], F32R,
                           kind="ExternalInput").ap()
    out = nc.dram_tensor("out", [10, bc], F32, kind="ExternalOutput").ap()

    with tile.TileContext(nc) as tc:
        with tc.tile_pool(name="wpool", bufs=1) as wpool, \
             tc.tile_pool(name="sb", bufs=1) as sb, \
             tc.tile_pool(name="ps", bufs=1, space="PSUM") as ps:

            wb = wpool.tile([128, WBLOB_COLS], F32R, tag="wb")
            # conv1 weights first so the first matmuls aren't blocked on the
            # full blob transfer; the rest lands during conv1
            c1w = 4 * 128
            nc.scalar.dma_start(wb[:, 0:c1w], wblob[:, 0:c1w])
            nc.scalar.dma_start(wb[:, c1w:WBLOB_COLS],
                                wblob[:, c1w:WBLOB_COLS])

            def wr(p0, p1, c0, c1):  # f32r slice of the weight blob
                return wb[p0:p1, c0:c1]

            b1col = wb[0:128, C_B1:C_B1 + 1].bitcast(F32)
            b2col = [wb[0:100, C_B2[s]:C_B2[s] + 1].bitcast(F32)
                     for s in range(2)]
            bf1col = wb[0:50, C_BF1:C_BF1 + 1].bitcast(F32)
            bf2col = wb[0:10, C_BF2:C_BF2 + 1].bitcast(F32)

            # per-N-tile stage emitters -----------------------------------
            def conv1_stage(nt):
                """conv1 + 2x2 maxpool (bias+relu fused) -> 12 row-pair
                tiles r2[(q, h)] with partitions (row-in-pair, ch, col)."""
                n0 = nt * N
                r2 = {}
                for q in range(6):
                    for hh in range(2):
                        r2[q, hh] = sb.tile([128, N], F32R,
                                            tag=f"r2_{q}_{hh}", bufs=2,
                                            name=f"r2_{q}_{hh}_{nt}")
                ei = 0
                for t in range(6):
                    for h in range(2):
                        rhs = sb.tile([128, N], F32R, tag="rhs1",
                                      bufs=cfg["rhs_bufs"])
                        nc.sync.dma_start(
                            rhs[:], xT[4 * t:4 * t + 8, 12 * h:12 * h + 16,
                                       n0:n0 + N])
                        V = []
                        for j2 in range(2):
                            pa = ps.tile([128, N], F32, tag="p1e",
                                         bufs=cfg["p1_bufs"],
                                         name=f"p1e_{nt}_{t}_{h}_{j2}")
                            pb = ps.tile([128, N], F32, tag="p1o",
                                         bufs=cfg["p1_bufs"],
                                         name=f"p1o_{nt}_{t}_{h}_{j2}")
                            for par, pt in ((0, pa), (1, pb)):
                                co = C_LHST1[j2][par]
                                nc.tensor.matmul(pt[:],
                                                 wr(0, 128, co, co + 128),
                                                 rhs[:], start=True, stop=True)
                            # u = relu(Pa + b): the relu makes the final
                            # plain max-fold equal relu(pool+b)
                            u = sb.tile([128, N], F32, tag="u1", bufs=3)
                            if ei % cfg["cme_dve_mod"] == cfg["cme_dve_mod"] - 1:
                                nc.vector.tensor_scalar(u[:], pa[:], b1col,
                                                        0.0, Add, Max)
                            else:
                                nc.scalar.activation(u[:], pa[:], Relu,
                                                     bias=b1col)
                            # v = max(Pb + b, u): column pool (DVE, 1 PSUM)
                            v = sb.tile([128, N], F32, tag=f"v1_{j2}", bufs=2)
                            nc.vector.scalar_tensor_tensor(
                                v[:], pb[:], b1col, u[:], Add, Max)
                            V.append(v)
                            ei += 1
                        # row pool (relu already folded into u): equal-base
                        nc.vector.tensor_max(r2[t, h][:], V[0][:], V[1][:])
                return r2

            def conv2_fc1_stage(nt, r2):
                """conv2 + relu + fc1 accumulation (fc1 one group behind so
                the PE never waits on the relu engine)."""
                pfc1 = ps.tile([50, N], F32, tag="pfc1", bufs=1,
                               name=f"pfc1_{nt}")
                pending = []  # (a2_tile, fc1_col, first?) awaiting fc1 matmul
                gi = 0
                for yo in range(10):
                    for s in range(2):
                        p2 = ps.tile([100, N], F32, tag="p2", bufs=2,
                                     name=f"p2_{nt}_{yo}_{s}")
                        mi = 0
                        for pi in range(2):
                            for h in range(2):
                                co = C_LHST2[yo % 2][pi][s][h]
                                nc.tensor.matmul(
                                    p2[:], wr(0, 128, co, co + 100),
                                    r2[yo // 2 + pi, h][:],
                                    start=(mi == 0), stop=(mi == 3))
                                mi += 1
                        a2 = sb.tile([100, N], F32R, tag="a2", bufs=3,
                                     name=f"a2_{nt}_{yo}_{s}")
                        if (2 * yo + s) % cfg["a2_dve_mod"] == cfg["a2_dve_mod"] - 1:
                            nc.vector.tensor_scalar(a2[:], p2[:], b2col[s],
                                                    0.0, Add, Max)
                        else:
                            nc.scalar.activation(a2[:], p2[:], Relu,
                                                 bias=b2col[s])
                        pending.append((a2, C_LHSTF1[yo][s]))
                        if len(pending) > 1:
                            pa2, pcf = pending.pop(0)
                            nc.tensor.matmul(
                                pfc1[:], wr(0, 100, pcf, pcf + 50),
                                pa2[:],
                                start=(gi == 0), stop=False)
                            gi += 1
                pa2, pcf = pending.pop(0)
                nc.tensor.matmul(pfc1[:], wr(0, 100, pcf, pcf + 50),
                                 pa2[:], start=False, stop=True)
                return pfc1

            def tail_stage(nt, pfc1):
                """fc2 + log_softmax + output DMA."""
                n0 = nt * N
                a3 = sb.tile([50, N], F32R, tag="a3", bufs=2,
                             name=f"a3_{nt}")
                nc.scalar.activation(a3[:], pfc1[:], Relu, bias=bf1col)
                zps = ps.tile([10, N], F32, tag="ptail", bufs=1,
                              name=f"zps_{nt}")
                nc.tensor.matmul(zps[:], wr(0, 50, C_LHSTF2, C_LHSTF2 + 10),
                                 a3[:], start=True, stop=True)
                z = sb.tile([10, N], F32, tag="z", bufs=2, name=f"z_{nt}")
                nc.scalar.activation(z[:], zps[:], Ident, bias=bf2col,
                                     scale=Etot)
                ez = sb.tile([10, N], F32R, tag="ez", bufs=2,
                             name=f"ez_{nt}")
                nc.scalar.activation(ez[:], zps[:], Exp, bias=bf2col,
                                     scale=Etot)
                sps = ps.tile([1, N], F32, tag="ptail", bufs=1,
                               name=f"sps_{nt}")
                nc.tensor.matmul(sps[:], wr(0, 10, C_ONES_COL, C_ONES_COL + 1),
                                 ez[:], start=True, stop=True)
                lse = sb.tile([1, N], F32R, tag="lse", bufs=2,
                              name=f"lse_{nt}")
                nc.scalar.activation(lse[:], sps[:], Ln)
                bps = ps.tile([10, N], F32, tag="ptail", bufs=1,
                               name=f"bps_{nt}")
                nc.tensor.matmul(bps[:], wr(0, 1, C_ONES_ROW, C_ONES_ROW + 10),
                                 lse[:], start=True, stop=True)
                osb = sb.tile([10, N], F32, tag="osb", bufs=2, name=f"osb_{nt}")
                nc.vector.tensor_sub(osb[:], z[:], bps[:])
                nc.sync.dma_start(out[:, n0:n0 + N], osb[:])

            # interleave N-tiles: tile k+1's conv1 is emitted before tile
            # k's tail so the PE stays dense across the serial softmax tail
            for _rep in range(repeat):
                r2s = {}
                for nt in range(n_tiles):
                    r2s[nt] = conv1_stage(nt)
                    if nt > 0:
                        k = nt - 1
                        tail_stage(k, conv2_fc1_stage(k, r2s.pop(k)))
                k = n_tiles - 1
                tail_stage(k, conv2_fc1_stage(k, r2s.pop(k)))
    nc.compile()
    return nc


def kernel(**inputs):
    global LAST_EXEC_TIME_NS, LAST_RESULTS
    x = np.ascontiguousarray(np.asarray(inputs["x"], dtype=np.float32))
    wb, scales = _host_prep({k: np.asarray(v) for k, v in inputs.items()})

    nc = build_program(scales)

    in_maps = []
    for i in range(N_CORES):
        xs = x[i * BC:(i + 1) * BC, 0]            # [BC, 28, 28]
        xTi = np.ascontiguousarray(xs.transpose(1, 2, 0))  # [28, 28, BC]
        in_maps.append({"xT": xTi, "wblob": wb})

    trace = bool(os.environ.get("KERNEL_TRACE"))
    res = run_bass_kernel_spmd(nc, in_maps, list(range(N_CORES)), trace=trace)
    LAST_EXEC_TIME_NS = res.exec_time_ns
    LAST_RESULTS = res

    out = np.empty((B_TOTAL, 10), np.float32)
    for i in range(N_CORES):
        out[i * BC:(i + 1) * BC] = res.results[i]["out"].T
    return out

